# revision 14
# baseline (speedup 1.0000x reference)
"""Multi-head attention (B=4, T=2048, C=1024, H=16) on 8 Trainium2 NeuronCores.

Sharding: core c handles batch c//2 and heads (c%2)*8 .. (c%2)*8+7 (tensor
parallel over heads x data parallel over batch). Each core computes a partial
output projection over its 512 head-dims; the host sums the two partials per
batch, transposes, and adds the bias.

Device-side layout (per core):
  xT  [1024, 2048]  x[b] transposed (host-prepped)
  wqT/wkT/wvT [1024, 512]   W[rows,:].T for this core's 8 heads
  wpT [512, 1024]           Wp[:, rows].T
  yT  [1024, 2048]          partial y[b].T (output)

All matmuls run as float32r (TF32-like, ~1 cycle/row for N>=256 vs 4 for
fp32). Scores are computed transposed (S^T tiles [128 Tk, 1024 Tq]) so the
softmax denominator comes free from a ones-column appended to V, and the
attention output lands directly in the [head-dim, T] layout the output
projection consumes. The phase-2 inner loop is software-pipelined (scores for
chunk tk+1 issue before the exp-gated AV of chunk tk) so the PE never stalls
long enough for the HAM clock gate to re-throttle it to 1.2 GHz.
"""

import numpy as np

B, T, C, H = 4, 2048, 1024, 16
D = C // H  # 64
N_CORES = 8
HPC = H // 2  # heads per core = 8
DIMS = HPC * D  # 512 local head dims per core

_cache = {}


def _build_nc():
    import concourse.bacc as bacc
    import concourse.tile as tile
    import concourse.mybir as mybir

    f32 = mybir.dt.float32
    f32r = mybir.dt.float32r
    Exp = mybir.ActivationFunctionType.Exp

    nc = bacc.Bacc("TRN2", target_bir_lowering=False, debug=False, num_devices=N_CORES)

    xT = nc.dram_tensor("xT", [C, T], f32r, kind="ExternalInput").ap()
    wqT = nc.dram_tensor("wqT", [C, DIMS], f32r, kind="ExternalInput").ap()
    wkT = nc.dram_tensor("wkT", [C, DIMS], f32r, kind="ExternalInput").ap()
    wvT = nc.dram_tensor("wvT", [C, DIMS], f32r, kind="ExternalInput").ap()
    wpT = nc.dram_tensor("wpT", [DIMS, C], f32r, kind="ExternalInput").ap()
    onesd = nc.dram_tensor("ones", [128, 128], f32r, kind="ExternalInput").ap()
    yT = nc.dram_tensor("yT", [C, T], f32, kind="ExternalOutput").ap()

    CC = C // 128  # 8 cin chunks
    NSTRIP = T // 512  # 4 strips for phase 1
    NTK = T // 128  # 16 key chunks
    NQB = T // 1024  # 2 query blocks
    DC = DIMS // 128  # 4 dim chunks
    OC = C // 128  # 8 output chunks

    xT_r = xT.rearrange("(c p) t -> p c t", p=128)
    yT_r = yT.rearrange("(o p) t -> p o t", p=128)

    with tile.TileContext(nc) as tc:
        with (
            tc.tile_pool(name="persist", bufs=1) as persist,
            tc.tile_pool(name="wpool", bufs=2) as wpool,
            tc.tile_pool(name="m8k", bufs=3) as m8k,
            tc.tile_pool(name="ptpool", bufs=3) as ptpool,
            tc.tile_pool(name="ycpool", bufs=2) as ycpool,
            tc.tile_pool(name="dpool", bufs=2, space="DRAM") as dpool,
            tc.tile_pool(name="ps_sc", bufs=2, space="PSUM") as ps_sc,
            tc.tile_pool(name="ps_av", bufs=2, space="PSUM") as ps_av,
        ):
            q_sb = persist.tile([128, DC, T], f32r, tag="q_sb")
            k_sb = persist.tile([128, DC, T], f32r, tag="k_sb")
            v_sb = persist.tile([128, NTK, HPC, D + 1], f32r, tag="v_sb")
            onT = persist.tile([128, DC, T], f32r, tag="onT")

            # ---------------- phase 1: q/k/v projections -------------------
            with nc.named_scope("phase1"):
                nc.sync.dma_start(
                    out=v_sb[:, :, :, D : D + 1],
                    in_=onesd.rearrange("p (a b) -> p a b", a=16)[:, :, :, None],
                )
                for tgt, wdram in (("q", wqT), ("k", wkT), ("v", wvT)):
                    w = wpool.tile([128, CC, DIMS], f32r, tag="w")
                    wdr = wdram.rearrange("(c p) m -> p c m", p=128)
                    nc.sync.dma_start(out=w[:, 0:2, :], in_=wdr[:, 0:2, :])
                    nc.sync.dma_start(out=w[:, 2:4, :], in_=wdr[:, 2:4, :])
                    nc.sync.dma_start(out=w[:, 4:6, :], in_=wdr[:, 4:6, :])
                    nc.sync.dma_start(out=w[:, 6:8, :], in_=wdr[:, 6:8, :])
                    for s in range(NSTRIP):
                        xa = m8k.tile([128, 4, 512], f32r, tag="m8k")
                        nc.sync.dma_start(
                            out=xa, in_=xT_r[:, 0:4, s * 512 : (s + 1) * 512]
                        )
                        xb = m8k.tile([128, 4, 512], f32r, tag="m8k")
                        nc.sync.dma_start(
                            out=xb, in_=xT_r[:, 4:8, s * 512 : (s + 1) * 512]
                        )
                        halves = (xa, xb)
                        # 4 interleaved accumulation chains (c-major) so the
                        # PE never serializes on one PSUM bank, and the xa/xb
                        # tiles are released early enough to prefetch the next
                        # strip.
                        chains = []
                        for j in range(4):
                            pool2 = ps_sc if j % 2 == 0 else ps_av
                            cps = pool2.tile(
                                [128, 1024], f32, tag=("sc" if j % 2 == 0 else "av")
                            )
                            chains.append(cps)
                        if tgt in ("q", "k"):
                            tgt_sb = q_sb if tgt == "q" else k_sb
                            for c in range(CC):
                                for dc in range(DC):
                                    nc.tensor.matmul(
                                        chains[dc][:, 0:512],
                                        w[:, c, dc * 128 : (dc + 1) * 128],
                                        halves[c // 4][:, c % 4, :],
                                        start=(c == 0),
                                        stop=(c == CC - 1),
                                        skip_group_check=True,
                                    )
                            for dc in range(DC):
                                if (s + dc) % 2:
                                    nc.vector.tensor_copy(
                                        out=tgt_sb[:, dc, s * 512 : (s + 1) * 512],
                                        in_=chains[dc][:, 0:512],
                                    )
                                else:
                                    nc.scalar.copy(
                                        out=tgt_sb[:, dc, s * 512 : (s + 1) * 512],
                                        in_=chains[dc][:, 0:512],
                                    )
                        else:
                            for c in range(CC):
                                for tc2 in range(4):
                                    nc.tensor.matmul(
                                        chains[tc2][:, 0:DIMS],
                                        halves[c // 4][
                                            :, c % 4, tc2 * 128 : (tc2 + 1) * 128
                                        ],
                                        w[:, c, :],
                                        start=(c == 0),
                                        stop=(c == CC - 1),
                                        skip_group_check=True,
                                    )
                            for tc2 in range(4):
                                tcg = s * 4 + tc2
                                if tc2 % 2:
                                    nc.vector.tensor_copy(
                                        out=v_sb[:, tcg, :, 0:D],
                                        in_=chains[tc2][:, 0:DIMS].rearrange(
                                            "p (h d) -> p h d", h=HPC
                                        ),
                                    )
                                else:
                                    nc.scalar.copy(
                                        out=v_sb[:, tcg, :, 0:D],
                                        in_=chains[tc2][:, 0:DIMS].rearrange(
                                            "p (h d) -> p h d", h=HPC
                                        ),
                                    )

            # load wp into the weight pool (overlaps the v pass / phase 2)
            wp_sb = wpool.tile([128, DC, C], f32r, tag="w")
            nc.sync.dma_start(out=wp_sb, in_=wpT.rearrange("(c p) n -> p c n", p=128))

            # ---------------- phase 2: attention, head pairs ----------------
            # Heads are processed in even/odd pairs sharing one 128-partition
            # chunk: the even head's K=64 score matmuls use array rows 0-63
            # and the odd head's rows 64-127, alternating, so the two run
            # concurrently in the PE (measured 2x). One exp covers both heads.
            with nc.named_scope("phase2"):
                for p in range(HPC // 2):
                    he, ho = 2 * p, 2 * p + 1
                    rb_e = m8k.tile([128, T], f32, tag="m8k")
                    rb_o = m8k.tile([128, T], f32, tag="m8k")
                    for qb in range(NQB):
                        po_e = ps_av.tile([128, 1024], f32, tag="av")
                        po_o = ps_av.tile([128, 1024], f32, tag="av")

                        def scores(u):
                            s2 = u % 2
                            strip = qb * 2 + s2
                            tk = u // 2
                            ps2 = ps_sc.tile([128, 1024], f32, tag="sc")
                            for half, hp in ((0, 0), (1, 64)):
                                nc.tensor.matmul(
                                    ps2[:, half * 512 : (half + 1) * 512],
                                    k_sb[hp : hp + 64, p, tk * 128 : (tk + 1) * 128],
                                    q_sb[hp : hp + 64, p, strip * 512 : (strip + 1) * 512],
                                    start=True,
                                    stop=True,
                                    skip_group_check=True,
                                )
                            return ps2

                        NU = 2 * NTK
                        cur = scores(0)
                        for u in range(NU):
                            s2 = u % 2
                            tk = u // 2
                            pt = ptpool.tile([128, 1024], f32r, tag="pt")
                            nc.scalar.activation(
                                out=pt[:], in_=cur[:], func=Exp, scale=0.125
                            )
                            if u + 1 < NU:
                                nxt = scores(u + 1)
                            for po, hl in ((po_e, he), (po_o, ho)):
                                nc.tensor.matmul(
                                    po[0:65, s2 * 512 : (s2 + 1) * 512],
                                    v_sb[:, tk, hl, :],
                                    pt[:, (hl % 2) * 512 : (hl % 2) * 512 + 512],
                                    start=(tk == 0),
                                    stop=(tk == NTK - 1),
                                    skip_group_check=True,
                                )
                            if u + 1 < NU:
                                cur = nxt
                        nc.vector.tensor_copy(
                            out=onT[0:64, p, qb * 1024 : (qb + 1) * 1024],
                            in_=po_e[0:64, :],
                        )
                        nc.vector.tensor_copy(
                            out=rb_e[64:65, qb * 1024 : (qb + 1) * 1024],
                            in_=po_e[64:65, :],
                        )
                        tmp = ycpool.tile([128, 1024], f32r, tag="yc")
                        nc.vector.tensor_copy(out=tmp[0:64, :], in_=po_o[0:64, :])
                        nc.sync.dma_start(
                            out=onT[64:128, p, qb * 1024 : (qb + 1) * 1024],
                            in_=tmp[0:64, :],
                        )
                        nc.vector.tensor_copy(
                            out=rb_o[64:65, qb * 1024 : (qb + 1) * 1024],
                            in_=po_o[64:65, :],
                        )
                    # reciprocal of the softmax denominators, broadcast to 64
                    # partitions via a DRAM bounce (SBUF DMA can't step-0 on
                    # the partition dim; DRAM-side APs can).
                    for rb, lo in ((rb_e, 0), (rb_o, 64)):
                        dscr = dpool.tile([1, T], f32, tag="dscr")
                        nc.sync.dma_start(out=dscr[:], in_=rb[64:65, :])
                        nc.sync.dma_start(
                            out=rb[0:64, :], in_=dscr.to_broadcast([64, T])
                        )
                        # custom DVE ops only work at base partition 0
                        nc.vector.reciprocal_approx_fast(
                            out=rb[0:64, :], in_=rb[0:64, :]
                        )
                        if lo:
                            nc.sync.dma_start(out=rb[64:128, :], in_=rb[0:64, :])
                        nc.vector.tensor_mul(
                            onT[lo : lo + 64, p, :],
                            onT[lo : lo + 64, p, :],
                            rb[lo : lo + 64, :],
                        )

            # ---------------- phase 3: output projection -------------------
            with nc.named_scope("phase3"):
                for oc in range(OC):
                    py0 = ps_sc.tile([128, 1024], f32, tag="sc")
                    py1 = ps_av.tile([128, 1024], f32, tag="av")
                    pys = (py0, py1)
                    for c in range(DC):
                        for half in range(2):
                            for s2 in range(2):
                                strip = half * 2 + s2
                                nc.tensor.matmul(
                                    pys[half][:, s2 * 512 : (s2 + 1) * 512],
                                    wp_sb[:, c, oc * 128 : (oc + 1) * 128],
                                    onT[:, c, strip * 512 : (strip + 1) * 512],
                                    start=(c == 0),
                                    stop=(c == DC - 1),
                                    skip_group_check=True,
                                )
                    for half in range(2):
                        yc = ycpool.tile([128, 1024], f32, tag="yc")
                        if (oc + half) % 2:
                            nc.vector.tensor_copy(out=yc[:], in_=pys[half][:])
                        else:
                            nc.scalar.copy(out=yc[:], in_=pys[half][:])
                        nc.sync.dma_start(
                            out=yT_r[:, oc, half * 1024 : (half + 1) * 1024],
                            in_=yc[:],
                        )

    nc.compile()
    return nc


def _get_nc():
    if "nc" not in _cache:
        _cache["nc"] = _build_nc()
    return _cache["nc"]


def kernel(x, Wk, Wq, Wv, Wp, bp):
    from concourse.bass_utils import run_bass_kernel_spmd

    x = np.asarray(x, dtype=np.float32)
    Wk = np.asarray(Wk, dtype=np.float32)
    Wq = np.asarray(Wq, dtype=np.float32)
    Wv = np.asarray(Wv, dtype=np.float32)
    Wp = np.asarray(Wp, dtype=np.float32)
    bp = np.asarray(bp, dtype=np.float32)

    nc = _get_nc()

    ins = []
    for c in range(N_CORES):
        b, j = c // 2, c % 2
        rows = slice(j * DIMS, (j + 1) * DIMS)
        ins.append(
            {
                "xT": np.ascontiguousarray(x[b].T),
                "wqT": np.ascontiguousarray(Wq[rows, :].T),
                "wkT": np.ascontiguousarray(Wk[rows, :].T),
                "wvT": np.ascontiguousarray(Wv[rows, :].T),
                "wpT": np.ascontiguousarray(Wp[:, rows].T),
                "ones": np.ones((128, 128), np.float32),
            }
        )

    res = run_bass_kernel_spmd(
        nc, ins, core_ids=list(range(N_CORES)), **_cache.get("run_kwargs", {})
    )
    _cache["last_result"] = res

    y = np.empty((B, T, C), np.float32)
    for b in range(B):
        yTp = res.results[2 * b]["yT"] + res.results[2 * b + 1]["yT"]
        y[b] = yTp.T + bp
    return y


# revision 15
# speedup vs baseline: 1.0016x; 1.0016x over previous
"""Multi-head attention (B=4, T=2048, C=1024, H=16) on 8 Trainium2 NeuronCores.

Sharding: core c handles batch c//2 and heads (c%2)*8 .. (c%2)*8+7 (tensor
parallel over heads x data parallel over batch). Each core computes a partial
output projection over its 512 head-dims; the host sums the two partials per
batch, transposes, and adds the bias.

Device-side layout (per core):
  xT  [1024, 2048]  x[b] transposed (host-prepped)
  wqT/wkT/wvT [1024, 512]   W[rows,:].T for this core's 8 heads
  wpT [512, 1024]           Wp[:, rows].T
  yT  [1024, 2048]          partial y[b].T (output)

All matmuls run as float32r (TF32-like, ~1 cycle/row for N>=256 vs 4 for
fp32). Scores are computed transposed (S^T tiles [128 Tk, 1024 Tq]) so the
softmax denominator comes free from a ones-column appended to V, and the
attention output lands directly in the [head-dim, T] layout the output
projection consumes. The phase-2 inner loop is software-pipelined (scores for
chunk tk+1 issue before the exp-gated AV of chunk tk) so the PE never stalls
long enough for the HAM clock gate to re-throttle it to 1.2 GHz.
"""

import numpy as np

B, T, C, H = 4, 2048, 1024, 16
D = C // H  # 64
N_CORES = 8
HPC = H // 2  # heads per core = 8
DIMS = HPC * D  # 512 local head dims per core

_cache = {}


def _build_nc():
    import concourse.bacc as bacc
    import concourse.tile as tile
    import concourse.mybir as mybir

    f32 = mybir.dt.float32
    f32r = mybir.dt.float32r
    Exp = mybir.ActivationFunctionType.Exp

    nc = bacc.Bacc("TRN2", target_bir_lowering=False, debug=False, num_devices=N_CORES)

    xT = nc.dram_tensor("xT", [C, T], f32r, kind="ExternalInput").ap()
    wqT = nc.dram_tensor("wqT", [C, DIMS], f32r, kind="ExternalInput").ap()
    wkT = nc.dram_tensor("wkT", [C, DIMS], f32r, kind="ExternalInput").ap()
    wvT = nc.dram_tensor("wvT", [C, DIMS], f32r, kind="ExternalInput").ap()
    wpT = nc.dram_tensor("wpT", [DIMS, C], f32r, kind="ExternalInput").ap()
    onesd = nc.dram_tensor("ones", [128, 128], f32r, kind="ExternalInput").ap()
    yT = nc.dram_tensor("yT", [C, T], f32, kind="ExternalOutput").ap()

    CC = C // 128  # 8 cin chunks
    NSTRIP = T // 512  # 4 strips for phase 1
    NTK = T // 128  # 16 key chunks
    NQB = T // 1024  # 2 query blocks
    DC = DIMS // 128  # 4 dim chunks
    OC = C // 128  # 8 output chunks

    xT_r = xT.rearrange("(c p) t -> p c t", p=128)
    yT_r = yT.rearrange("(o p) t -> p o t", p=128)

    with tile.TileContext(nc) as tc:
        with (
            tc.tile_pool(name="persist", bufs=1) as persist,
            tc.tile_pool(name="wpool", bufs=2) as wpool,
            tc.tile_pool(name="m8k", bufs=3) as m8k,
            tc.tile_pool(name="ptpool", bufs=3) as ptpool,
            tc.tile_pool(name="ycpool", bufs=2) as ycpool,
            tc.tile_pool(name="dpool", bufs=2, space="DRAM") as dpool,
            tc.tile_pool(name="ps_sc", bufs=2, space="PSUM") as ps_sc,
            tc.tile_pool(name="ps_av", bufs=2, space="PSUM") as ps_av,
        ):
            q_sb = persist.tile([128, DC, T], f32r, tag="q_sb")
            k_sb = persist.tile([128, DC, T], f32r, tag="k_sb")
            v_sb = persist.tile([128, NTK, HPC, D + 1], f32r, tag="v_sb")
            onT = persist.tile([128, DC, T], f32r, tag="onT")

            # ---------------- phase 1: q/k/v projections -------------------
            with nc.named_scope("phase1"):
                nc.sync.dma_start(
                    out=v_sb[:, :, :, D : D + 1],
                    in_=onesd.rearrange("p (a b) -> p a b", a=16)[:, :, :, None],
                )
                for tgt, wdram in (("q", wqT), ("k", wkT), ("v", wvT)):
                    w = wpool.tile([128, CC, DIMS], f32r, tag="w")
                    wdr = wdram.rearrange("(c p) m -> p c m", p=128)
                    nc.sync.dma_start(out=w[:, 0:2, :], in_=wdr[:, 0:2, :])
                    nc.sync.dma_start(out=w[:, 2:4, :], in_=wdr[:, 2:4, :])
                    nc.sync.dma_start(out=w[:, 4:6, :], in_=wdr[:, 4:6, :])
                    nc.sync.dma_start(out=w[:, 6:8, :], in_=wdr[:, 6:8, :])
                    for s in range(NSTRIP):
                        xa = m8k.tile([128, 4, 512], f32r, tag="m8k")
                        nc.sync.dma_start(
                            out=xa, in_=xT_r[:, 0:4, s * 512 : (s + 1) * 512]
                        )
                        xb = m8k.tile([128, 4, 512], f32r, tag="m8k")
                        nc.sync.dma_start(
                            out=xb, in_=xT_r[:, 4:8, s * 512 : (s + 1) * 512]
                        )
                        halves = (xa, xb)
                        # 4 interleaved accumulation chains (c-major) so the
                        # PE never serializes on one PSUM bank, and the xa/xb
                        # tiles are released early enough to prefetch the next
                        # strip.
                        chains = []
                        for j in range(4):
                            pool2 = ps_sc if j % 2 == 0 else ps_av
                            cps = pool2.tile(
                                [128, 1024], f32, tag=("sc" if j % 2 == 0 else "av")
                            )
                            chains.append(cps)
                        if tgt in ("q", "k"):
                            tgt_sb = q_sb if tgt == "q" else k_sb
                            for c in range(CC):
                                for dc in range(DC):
                                    nc.tensor.matmul(
                                        chains[dc][:, 0:512],
                                        w[:, c, dc * 128 : (dc + 1) * 128],
                                        halves[c // 4][:, c % 4, :],
                                        start=(c == 0),
                                        stop=(c == CC - 1),
                                        skip_group_check=True,
                                    )
                            for dc in range(DC):
                                if (s + dc) % 2:
                                    nc.vector.tensor_copy(
                                        out=tgt_sb[:, dc, s * 512 : (s + 1) * 512],
                                        in_=chains[dc][:, 0:512],
                                    )
                                else:
                                    nc.scalar.copy(
                                        out=tgt_sb[:, dc, s * 512 : (s + 1) * 512],
                                        in_=chains[dc][:, 0:512],
                                    )
                        else:
                            for c in range(CC):
                                for tc2 in range(4):
                                    nc.tensor.matmul(
                                        chains[tc2][:, 0:DIMS],
                                        halves[c // 4][
                                            :, c % 4, tc2 * 128 : (tc2 + 1) * 128
                                        ],
                                        w[:, c, :],
                                        start=(c == 0),
                                        stop=(c == CC - 1),
                                        skip_group_check=True,
                                    )
                            for tc2 in range(4):
                                tcg = s * 4 + tc2
                                if tc2 % 2:
                                    nc.vector.tensor_copy(
                                        out=v_sb[:, tcg, :, 0:D],
                                        in_=chains[tc2][:, 0:DIMS].rearrange(
                                            "p (h d) -> p h d", h=HPC
                                        ),
                                    )
                                else:
                                    nc.scalar.copy(
                                        out=v_sb[:, tcg, :, 0:D],
                                        in_=chains[tc2][:, 0:DIMS].rearrange(
                                            "p (h d) -> p h d", h=HPC
                                        ),
                                    )

            # load wp into the weight pool (overlaps the v pass / phase 2)
            wp_sb = wpool.tile([128, DC, C], f32r, tag="w")
            nc.sync.dma_start(out=wp_sb, in_=wpT.rearrange("(c p) n -> p c n", p=128))

            # ---------------- phase 2: attention, head pairs ----------------
            # Heads are processed in even/odd pairs sharing one 128-partition
            # chunk: the even head's K=64 score matmuls use array rows 0-63
            # and the odd head's rows 64-127, alternating, so the two run
            # concurrently in the PE (measured 2x). One exp covers both heads.
            with nc.named_scope("phase2"):
                for p in range(HPC // 2):
                    he, ho = 2 * p, 2 * p + 1
                    rb_e = m8k.tile([128, T], f32, tag="m8k")
                    rb_o = m8k.tile([128, T], f32, tag="m8k")
                    tmps = []
                    for qb in range(NQB):
                        po_e = ps_av.tile([128, 1024], f32, tag="av")
                        po_o = ps_av.tile([128, 1024], f32, tag="av")

                        def scores(u):
                            s2 = u % 2
                            strip = qb * 2 + s2
                            tk = u // 2
                            ps2 = ps_sc.tile([128, 1024], f32, tag="sc")
                            for half, hp in ((0, 0), (1, 64)):
                                nc.tensor.matmul(
                                    ps2[:, half * 512 : (half + 1) * 512],
                                    k_sb[hp : hp + 64, p, tk * 128 : (tk + 1) * 128],
                                    q_sb[hp : hp + 64, p, strip * 512 : (strip + 1) * 512],
                                    start=True,
                                    stop=True,
                                    skip_group_check=True,
                                )
                            return ps2

                        NU = 2 * NTK
                        cur = scores(0)
                        for u in range(NU):
                            s2 = u % 2
                            tk = u // 2
                            pt = ptpool.tile([128, 1024], f32r, tag="pt")
                            nc.scalar.activation(
                                out=pt[:], in_=cur[:], func=Exp, scale=0.125
                            )
                            if u + 1 < NU:
                                nxt = scores(u + 1)
                            for po, hl in ((po_e, he), (po_o, ho)):
                                nc.tensor.matmul(
                                    po[0:65, s2 * 512 : (s2 + 1) * 512],
                                    v_sb[:, tk, hl, :],
                                    pt[:, (hl % 2) * 512 : (hl % 2) * 512 + 512],
                                    start=(tk == 0),
                                    stop=(tk == NTK - 1),
                                    skip_group_check=True,
                                )
                            if u + 1 < NU:
                                cur = nxt
                        nc.vector.tensor_copy(
                            out=onT[0:64, p, qb * 1024 : (qb + 1) * 1024],
                            in_=po_e[0:64, :],
                        )
                        nc.vector.tensor_copy(
                            out=rb_e[64:65, qb * 1024 : (qb + 1) * 1024],
                            in_=po_e[64:65, :],
                        )
                        tmp = ycpool.tile([128, 1024], f32r, tag="yc")
                        nc.vector.tensor_copy(out=tmp[0:64, :], in_=po_o[0:64, :])
                        tmps.append(tmp)
                        nc.vector.tensor_copy(
                            out=rb_o[64:65, qb * 1024 : (qb + 1) * 1024],
                            in_=po_o[64:65, :],
                        )
                    # reciprocal of the softmax denominators, broadcast to 64
                    # partitions via a DRAM bounce (SBUF DMA can't step-0 on
                    # the partition dim; DRAM-side APs can).
                    for rb in (rb_e, rb_o):
                        dscr = dpool.tile([1, T], f32, tag="dscr")
                        nc.sync.dma_start(out=dscr[:], in_=rb[64:65, :])
                        nc.sync.dma_start(
                            out=rb[0:64, :], in_=dscr.to_broadcast([64, T])
                        )
                        # custom DVE ops only work at base partition 0
                        nc.vector.reciprocal_approx_fast(
                            out=rb[0:64, :], in_=rb[0:64, :]
                        )
                    nc.vector.tensor_mul(
                        onT[0:64, p, :], onT[0:64, p, :], rb_e[0:64, :]
                    )
                    for qb, tmp in enumerate(tmps):
                        nc.vector.tensor_mul(
                            tmp[0:64, :],
                            tmp[0:64, :],
                            rb_o[0:64, qb * 1024 : (qb + 1) * 1024],
                        )
                        nc.sync.dma_start(
                            out=onT[64:128, p, qb * 1024 : (qb + 1) * 1024],
                            in_=tmp[0:64, :],
                        )

            # ---------------- phase 3: output projection -------------------
            with nc.named_scope("phase3"):
                for oc in range(OC):
                    py0 = ps_sc.tile([128, 1024], f32, tag="sc")
                    py1 = ps_av.tile([128, 1024], f32, tag="av")
                    pys = (py0, py1)
                    for c in range(DC):
                        for half in range(2):
                            for s2 in range(2):
                                strip = half * 2 + s2
                                nc.tensor.matmul(
                                    pys[half][:, s2 * 512 : (s2 + 1) * 512],
                                    wp_sb[:, c, oc * 128 : (oc + 1) * 128],
                                    onT[:, c, strip * 512 : (strip + 1) * 512],
                                    start=(c == 0),
                                    stop=(c == DC - 1),
                                    skip_group_check=True,
                                )
                    for half in range(2):
                        yc = ycpool.tile([128, 1024], f32, tag="yc")
                        if (oc + half) % 2:
                            nc.vector.tensor_copy(out=yc[:], in_=pys[half][:])
                        else:
                            nc.scalar.copy(out=yc[:], in_=pys[half][:])
                        nc.sync.dma_start(
                            out=yT_r[:, oc, half * 1024 : (half + 1) * 1024],
                            in_=yc[:],
                        )

    nc.compile()
    return nc


def _get_nc():
    if "nc" not in _cache:
        _cache["nc"] = _build_nc()
    return _cache["nc"]


def kernel(x, Wk, Wq, Wv, Wp, bp):
    from concourse.bass_utils import run_bass_kernel_spmd

    x = np.asarray(x, dtype=np.float32)
    Wk = np.asarray(Wk, dtype=np.float32)
    Wq = np.asarray(Wq, dtype=np.float32)
    Wv = np.asarray(Wv, dtype=np.float32)
    Wp = np.asarray(Wp, dtype=np.float32)
    bp = np.asarray(bp, dtype=np.float32)

    nc = _get_nc()

    ins = []
    for c in range(N_CORES):
        b, j = c // 2, c % 2
        rows = slice(j * DIMS, (j + 1) * DIMS)
        ins.append(
            {
                "xT": np.ascontiguousarray(x[b].T),
                "wqT": np.ascontiguousarray(Wq[rows, :].T),
                "wkT": np.ascontiguousarray(Wk[rows, :].T),
                "wvT": np.ascontiguousarray(Wv[rows, :].T),
                "wpT": np.ascontiguousarray(Wp[:, rows].T),
                "ones": np.ones((128, 128), np.float32),
            }
        )

    res = run_bass_kernel_spmd(
        nc, ins, core_ids=list(range(N_CORES)), **_cache.get("run_kwargs", {})
    )
    _cache["last_result"] = res

    y = np.empty((B, T, C), np.float32)
    for b in range(B):
        yTp = res.results[2 * b]["yT"] + res.results[2 * b + 1]["yT"]
        y[b] = yTp.T + bp
    return y


# revision 16
# speedup vs baseline: 1.0262x; 1.0246x over previous
"""Multi-head attention (B=4, T=2048, C=1024, H=16) on 8 Trainium2 NeuronCores.

Sharding: core c handles batch c//2 and heads (c%2)*8 .. (c%2)*8+7 (tensor
parallel over heads x data parallel over batch). Each core computes a partial
output projection over its 512 head-dims; the host sums the two partials per
batch, transposes, and adds the bias.

Device-side layout (per core):
  xT  [1024, 2048]  x[b] transposed (host-prepped)
  wqT/wkT/wvT [1024, 512]   W[rows,:].T for this core's 8 heads
  wpT [512, 1024]           Wp[:, rows].T
  yT  [1024, 2048]          partial y[b].T (output)

All matmuls run as float32r (TF32-like, ~1 cycle/row for N>=256 vs 4 for
fp32). Scores are computed transposed (S^T tiles [128 Tk, 1024 Tq]) so the
softmax denominator comes free from a ones-column appended to V, and the
attention output lands directly in the [head-dim, T] layout the output
projection consumes. The phase-2 inner loop is software-pipelined (scores for
chunk tk+1 issue before the exp-gated AV of chunk tk) so the PE never stalls
long enough for the HAM clock gate to re-throttle it to 1.2 GHz.
"""

import numpy as np

B, T, C, H = 4, 2048, 1024, 16
D = C // H  # 64
N_CORES = 8
HPC = H // 2  # heads per core = 8
DIMS = HPC * D  # 512 local head dims per core

_cache = {}


def _build_nc():
    import concourse.bacc as bacc
    import concourse.tile as tile
    import concourse.mybir as mybir

    f32 = mybir.dt.float32
    f32r = mybir.dt.float32r
    Exp = mybir.ActivationFunctionType.Exp

    nc = bacc.Bacc("TRN2", target_bir_lowering=False, debug=False, num_devices=N_CORES)

    xT = nc.dram_tensor("xT", [C, T], f32r, kind="ExternalInput").ap()
    wqT = nc.dram_tensor("wqT", [C, DIMS], f32r, kind="ExternalInput").ap()
    wkT = nc.dram_tensor("wkT", [C, DIMS], f32r, kind="ExternalInput").ap()
    wvT = nc.dram_tensor("wvT", [C, DIMS], f32r, kind="ExternalInput").ap()
    wpT = nc.dram_tensor("wpT", [DIMS, C], f32r, kind="ExternalInput").ap()
    onesd = nc.dram_tensor("ones", [128, 128], f32r, kind="ExternalInput").ap()
    yT = nc.dram_tensor("yT", [C, T], f32, kind="ExternalOutput").ap()

    CC = C // 128  # 8 cin chunks
    NSTRIP = T // 512  # 4 strips for phase 1
    NTK = T // 128  # 16 key chunks
    NQB = T // 1024  # 2 query blocks
    DC = DIMS // 128  # 4 dim chunks
    OC = C // 128  # 8 output chunks

    xT_r = xT.rearrange("(c p) t -> p c t", p=128)
    yT_r = yT.rearrange("(o p) t -> p o t", p=128)

    with tile.TileContext(nc) as tc:
        with (
            tc.tile_pool(name="persist", bufs=1) as persist,
            tc.tile_pool(name="wpool", bufs=2) as wpool,
            tc.tile_pool(name="m8k", bufs=3) as m8k,
            tc.tile_pool(name="ptpool", bufs=3) as ptpool,
            tc.tile_pool(name="ycpool", bufs=2) as ycpool,
            tc.tile_pool(name="dpool", bufs=2, space="DRAM") as dpool,
            tc.tile_pool(name="ps_sc", bufs=2, space="PSUM") as ps_sc,
            tc.tile_pool(name="ps_av", bufs=2, space="PSUM") as ps_av,
        ):
            q_sb = persist.tile([128, DC, T], f32r, tag="q_sb")
            k_sb = persist.tile([128, DC, T], f32r, tag="k_sb")
            v_sb = persist.tile([128, NTK, HPC, D + 1], f32r, tag="v_sb")
            onT = persist.tile([128, DC, T], f32r, tag="onT")

            # ---------------- phase 1: q/k/v projections -------------------
            with nc.named_scope("phase1"):
                nc.sync.dma_start(
                    out=v_sb[:, :, :, D : D + 1],
                    in_=onesd.rearrange("p (a b) -> p a b", a=16)[:, :, :, None],
                )
                for tgt, wdram in (("q", wqT), ("k", wkT), ("v", wvT)):
                    w = wpool.tile([128, CC, DIMS], f32r, tag="w")
                    nc.sync.dma_start(
                        out=w, in_=wdram.rearrange("(c p) m -> p c m", p=128)
                    )
                    for s in range(NSTRIP):
                        xa = m8k.tile([128, 4, 512], f32r, tag="m8k")
                        nc.sync.dma_start(
                            out=xa, in_=xT_r[:, 0:4, s * 512 : (s + 1) * 512]
                        )
                        xb = m8k.tile([128, 4, 512], f32r, tag="m8k")
                        nc.sync.dma_start(
                            out=xb, in_=xT_r[:, 4:8, s * 512 : (s + 1) * 512]
                        )
                        halves = (xa, xb)
                        # 4 interleaved accumulation chains (c-major) so the
                        # PE never serializes on one PSUM bank, and the xa/xb
                        # tiles are released early enough to prefetch the next
                        # strip.
                        chains = []
                        for j in range(4):
                            pool2 = ps_sc if j % 2 == 0 else ps_av
                            cps = pool2.tile(
                                [128, 1024], f32, tag=("sc" if j % 2 == 0 else "av")
                            )
                            chains.append(cps)
                        if tgt in ("q", "k"):
                            tgt_sb = q_sb if tgt == "q" else k_sb
                            for c in range(CC):
                                for dc in range(DC):
                                    nc.tensor.matmul(
                                        chains[dc][:, 0:512],
                                        w[:, c, dc * 128 : (dc + 1) * 128],
                                        halves[c // 4][:, c % 4, :],
                                        start=(c == 0),
                                        stop=(c == CC - 1),
                                        skip_group_check=True,
                                    )
                            for dc in range(DC):
                                if (s + dc) % 2:
                                    nc.vector.tensor_copy(
                                        out=tgt_sb[:, dc, s * 512 : (s + 1) * 512],
                                        in_=chains[dc][:, 0:512],
                                    )
                                else:
                                    nc.scalar.copy(
                                        out=tgt_sb[:, dc, s * 512 : (s + 1) * 512],
                                        in_=chains[dc][:, 0:512],
                                    )
                        else:
                            for c in range(CC):
                                for tc2 in range(4):
                                    nc.tensor.matmul(
                                        chains[tc2][:, 0:DIMS],
                                        halves[c // 4][
                                            :, c % 4, tc2 * 128 : (tc2 + 1) * 128
                                        ],
                                        w[:, c, :],
                                        start=(c == 0),
                                        stop=(c == CC - 1),
                                        skip_group_check=True,
                                    )
                            for tc2 in range(4):
                                tcg = s * 4 + tc2
                                if tc2 % 2:
                                    nc.vector.tensor_copy(
                                        out=v_sb[:, tcg, :, 0:D],
                                        in_=chains[tc2][:, 0:DIMS].rearrange(
                                            "p (h d) -> p h d", h=HPC
                                        ),
                                    )
                                else:
                                    nc.scalar.copy(
                                        out=v_sb[:, tcg, :, 0:D],
                                        in_=chains[tc2][:, 0:DIMS].rearrange(
                                            "p (h d) -> p h d", h=HPC
                                        ),
                                    )

            # load wp into the weight pool (overlaps the v pass / phase 2)
            wp_sb = wpool.tile([128, DC, C], f32r, tag="w")
            nc.sync.dma_start(out=wp_sb, in_=wpT.rearrange("(c p) n -> p c n", p=128))

            # ---------------- phase 2: attention, head pairs ----------------
            # Heads are processed in even/odd pairs sharing one 128-partition
            # chunk: the even head's K=64 score matmuls use array rows 0-63
            # and the odd head's rows 64-127, alternating, so the two run
            # concurrently in the PE (measured 2x). One exp covers both heads.
            with nc.named_scope("phase2"):
                for p in range(HPC // 2):
                    he, ho = 2 * p, 2 * p + 1
                    rb_e = m8k.tile([128, T], f32, tag="m8k")
                    rb_o = m8k.tile([128, T], f32, tag="m8k")
                    tmps = []
                    for qb in range(NQB):
                        po_e = ps_av.tile([128, 1024], f32, tag="av")
                        po_o = ps_av.tile([128, 1024], f32, tag="av")

                        def scores(u):
                            s2 = u % 2
                            strip = qb * 2 + s2
                            tk = u // 2
                            ps2 = ps_sc.tile([128, 1024], f32, tag="sc")
                            for half, hp in ((0, 0), (1, 64)):
                                nc.tensor.matmul(
                                    ps2[:, half * 512 : (half + 1) * 512],
                                    k_sb[hp : hp + 64, p, tk * 128 : (tk + 1) * 128],
                                    q_sb[hp : hp + 64, p, strip * 512 : (strip + 1) * 512],
                                    start=True,
                                    stop=True,
                                    skip_group_check=True,
                                )
                            return ps2

                        NU = 2 * NTK
                        cur = scores(0)
                        for u in range(NU):
                            s2 = u % 2
                            tk = u // 2
                            pt = ptpool.tile([128, 1024], f32r, tag="pt")
                            nc.scalar.activation(
                                out=pt[:], in_=cur[:], func=Exp, scale=0.125
                            )
                            if u + 1 < NU:
                                nxt = scores(u + 1)
                            for po, hl in ((po_e, he), (po_o, ho)):
                                nc.tensor.matmul(
                                    po[0:65, s2 * 512 : (s2 + 1) * 512],
                                    v_sb[:, tk, hl, :],
                                    pt[:, (hl % 2) * 512 : (hl % 2) * 512 + 512],
                                    start=(tk == 0),
                                    stop=(tk == NTK - 1),
                                    skip_group_check=True,
                                )
                            if u + 1 < NU:
                                cur = nxt
                        nc.vector.tensor_copy(
                            out=onT[0:64, p, qb * 1024 : (qb + 1) * 1024],
                            in_=po_e[0:64, :],
                        )
                        nc.vector.tensor_copy(
                            out=rb_e[64:65, qb * 1024 : (qb + 1) * 1024],
                            in_=po_e[64:65, :],
                        )
                        tmp = ycpool.tile([128, 1024], f32r, tag="yc")
                        nc.vector.tensor_copy(out=tmp[0:64, :], in_=po_o[0:64, :])
                        tmps.append(tmp)
                        nc.vector.tensor_copy(
                            out=rb_o[64:65, qb * 1024 : (qb + 1) * 1024],
                            in_=po_o[64:65, :],
                        )
                    # reciprocal of the softmax denominators, broadcast to 64
                    # partitions via a DRAM bounce (SBUF DMA can't step-0 on
                    # the partition dim; DRAM-side APs can).
                    for rb in (rb_e, rb_o):
                        dscr = dpool.tile([1, T], f32, tag="dscr")
                        nc.sync.dma_start(out=dscr[:], in_=rb[64:65, :])
                        nc.sync.dma_start(
                            out=rb[0:64, :], in_=dscr.to_broadcast([64, T])
                        )
                        # custom DVE ops only work at base partition 0
                        nc.vector.reciprocal_approx_fast(
                            out=rb[0:64, :], in_=rb[0:64, :]
                        )
                    nc.vector.tensor_mul(
                        onT[0:64, p, :], onT[0:64, p, :], rb_e[0:64, :]
                    )
                    for qb, tmp in enumerate(tmps):
                        nc.vector.tensor_mul(
                            tmp[0:64, :],
                            tmp[0:64, :],
                            rb_o[0:64, qb * 1024 : (qb + 1) * 1024],
                        )
                        nc.sync.dma_start(
                            out=onT[64:128, p, qb * 1024 : (qb + 1) * 1024],
                            in_=tmp[0:64, :],
                        )

            # ---------------- phase 3: output projection -------------------
            with nc.named_scope("phase3"):
                for oc in range(OC):
                    py0 = ps_sc.tile([128, 1024], f32, tag="sc")
                    py1 = ps_av.tile([128, 1024], f32, tag="av")
                    pys = (py0, py1)
                    for c in range(DC):
                        for half in range(2):
                            for s2 in range(2):
                                strip = half * 2 + s2
                                nc.tensor.matmul(
                                    pys[half][:, s2 * 512 : (s2 + 1) * 512],
                                    wp_sb[:, c, oc * 128 : (oc + 1) * 128],
                                    onT[:, c, strip * 512 : (strip + 1) * 512],
                                    start=(c == 0),
                                    stop=(c == DC - 1),
                                    skip_group_check=True,
                                )
                    for half in range(2):
                        yc = ycpool.tile([128, 1024], f32, tag="yc")
                        if (oc + half) % 2:
                            nc.vector.tensor_copy(out=yc[:], in_=pys[half][:])
                        else:
                            nc.scalar.copy(out=yc[:], in_=pys[half][:])
                        nc.sync.dma_start(
                            out=yT_r[:, oc, half * 1024 : (half + 1) * 1024],
                            in_=yc[:],
                        )

    nc.compile()
    return nc


def _get_nc():
    if "nc" not in _cache:
        _cache["nc"] = _build_nc()
    return _cache["nc"]


def kernel(x, Wk, Wq, Wv, Wp, bp):
    from concourse.bass_utils import run_bass_kernel_spmd

    x = np.asarray(x, dtype=np.float32)
    Wk = np.asarray(Wk, dtype=np.float32)
    Wq = np.asarray(Wq, dtype=np.float32)
    Wv = np.asarray(Wv, dtype=np.float32)
    Wp = np.asarray(Wp, dtype=np.float32)
    bp = np.asarray(bp, dtype=np.float32)

    nc = _get_nc()

    ins = []
    for c in range(N_CORES):
        b, j = c // 2, c % 2
        rows = slice(j * DIMS, (j + 1) * DIMS)
        ins.append(
            {
                "xT": np.ascontiguousarray(x[b].T),
                "wqT": np.ascontiguousarray(Wq[rows, :].T),
                "wkT": np.ascontiguousarray(Wk[rows, :].T),
                "wvT": np.ascontiguousarray(Wv[rows, :].T),
                "wpT": np.ascontiguousarray(Wp[:, rows].T),
                "ones": np.ones((128, 128), np.float32),
            }
        )

    res = run_bass_kernel_spmd(
        nc, ins, core_ids=list(range(N_CORES)), **_cache.get("run_kwargs", {})
    )
    _cache["last_result"] = res

    y = np.empty((B, T, C), np.float32)
    for b in range(B):
        yTp = res.results[2 * b]["yT"] + res.results[2 * b + 1]["yT"]
        y[b] = yTp.T + bp
    return y
